# revision 10
# baseline (speedup 1.0000x reference)
"""Bahdanau additive attention on 8 TRN2 NeuronCores.

Problem shapes: encoder [4, 1024, 256], decoder [4, 256, 256],
W_a/U_a [256, 256], V_a [256, 1].
reference:
    enc_proj = enc @ W_a                  [B, E, H]
    dec_proj = dec @ U_a                  [B, D, H]
    score[b,d,e] = sum_h V[h] * tanh(dec_proj[b,d,h] + enc_proj[b,e,h])
    w = softmax(score, axis=-1)           [B, D, E]
    ctx = w @ enc                         [B, D, H]
    return (ctx, w)

Sharding: 8 cores = (batch b = core//2) x (decoder-row half = core%2).
Each core owns 128 decoder rows of one batch element; outputs are
disjoint so no collectives are needed.

Per-core dataflow (h on SBUF partitions, 2 chunks of 128):
  - TensorE: bf16 projections enc_projT[h,e], dec_projT[h,d]
    (contract over h_in on partitions; host ships transposed layouts).
  - VectorE: bf16 tensor_scalar pre-add (4x mode) builds
    arg[h, (r,c,e)] = enc_projT[h,e] + dec_projT[h,d].
  - ScalarE (the bottleneck, ~222us of 1-elem/cycle/lane tanh): ONE
    in-place tanh ACTIVATE per group of 8 decoder rows (FD=16384)
    amortizes the fixed ~224-cycle cost; groups stream back-to-back.
  - TensorE: score rows via the shifted-V trick: lhsT is a 128-col
    slice of a [128, 256] tensor whose only nonzero column (index 128)
    holds V; slice [128-d, 256-d) puts V in column d, so PSUM row d
    accumulates score[d, :] while other rows get += 0.  Interleaved
    FILLER matmuls (into a scratch bank) keep TensorE's HAM activity
    window busy so the real matmuls run at the warm 2.4 GHz rate and
    the last group's straggler tail stays small.
  - softmax along the free axis (exp + accum_out; max subtraction is
    skipped: |score| <= sum|V| ~ 10, far from fp32 overflow),
    TensorE transposes of w, bf16 context matmul, DMA out.
"""

import os
import sys

for _p in (
    "/opt/trn_rl_repo",
    "/root/.axon_site",
    "/root/.axon_site/_ro/trn_rl_repo",
    "/root/.axon_site/_ro/pypackages",
):
    if os.path.isdir(_p) and _p not in sys.path:
        sys.path.append(_p)

import ml_dtypes
import numpy as np

import concourse.mybir as mybir
from concourse import bacc, bass, tile

F32 = mybir.dt.float32
BF16 = mybir.dt.bfloat16

B, T_ENC, T_DEC, H = 4, 1024, 256, 256
P = 128  # SBUF partitions
HC = H // P  # h chunks (2)
DPC = 128  # decoder rows per core
N_CORES = 8

GROUPS = [4] + [8] * 15 + [2, 2]
assert sum(GROUPS) == DPC
GMAX = max(GROUPS)
# filler matmuls per group: pad PE work to ~the ACT group duration
# ACT group ~= (224 + G*2048)/1.2 ns; warm matmul ~= 218 ns (N=512)
FILLERS = {2: 8, 4: 16, 8: 32}

MODE = os.environ.get("ATTN_KERNEL_MODE", "v4")


def build_graph(mode=MODE):
    nc = bacc.Bacc("TRN2", target_bir_lowering=False, debug=False)

    # partition-major host layouts: one clean DMA per tensor
    enc_nat_d = nc.declare_dram_parameter("enc_nat", [P, 8, H], BF16, isOutput=False)
    encT_d = nc.declare_dram_parameter("encT", [P, HC, T_ENC], BF16, isOutput=False)
    decT_d = nc.declare_dram_parameter("decT", [P, HC, DPC], BF16, isOutput=False)
    W_d = nc.declare_dram_parameter("W", [P, HC, HC, P], BF16, isOutput=False)
    U_d = nc.declare_dram_parameter("U", [P, HC, HC, P], BF16, isOutput=False)
    Vbig_d = nc.declare_dram_parameter("Vbig", [P, HC, 2 * P], BF16, isOutput=False)
    ident_d = nc.declare_dram_parameter("ident", [P, P], F32, isOutput=False)
    w_out_d = nc.declare_dram_parameter("w_out", [DPC, T_ENC], F32, isOutput=True)
    ctx_out_d = nc.declare_dram_parameter("ctx_out", [DPC, H], F32, isOutput=True)

    TANH = mybir.ActivationFunctionType.Tanh
    EXP = mybir.ActivationFunctionType.Exp

    with tile.TileContext(nc) as tc:
        with (
            tc.tile_pool(name="const", bufs=1) as cpool,
            tc.tile_pool(name="psum_big", bufs=1, space="PSUM") as pbig,
            tc.tile_pool(name="psum_aux", bufs=1, space="PSUM") as paux,
            tc.tile_pool(name="work", bufs=1) as wpool,
            tc.tile_pool(name="epi", bufs=1) as epool,
        ):
            # ---------------- constants in ----------------
            # critical-path tensors go first, issued from gpsimd whose
            # preamble finishes before sync's; encT split for 2-queue overlap
            encT_sb = cpool.tile([P, HC, T_ENC], BF16, tag="encT_sb")
            for c in range(HC):
                nc.gpsimd.dma_start(out=encT_sb[:, c, :], in_=encT_d[:, c, :])
            W_sb = cpool.tile([P, HC, HC, P], BF16, tag="W_sb")
            nc.gpsimd.dma_start(out=W_sb[:], in_=W_d[:])
            U_sb = cpool.tile([P, HC, HC, P], BF16, tag="U_sb")
            nc.gpsimd.dma_start(out=U_sb[:], in_=U_d[:])
            decT_sb = cpool.tile([P, HC, DPC], BF16, tag="decT_sb")
            nc.gpsimd.dma_start(out=decT_sb[:], in_=decT_d[:])
            Vbig_sb = cpool.tile([P, HC, 2 * P], BF16, tag="Vbig_sb")
            nc.sync.dma_start(out=Vbig_sb[:], in_=Vbig_d[:])
            ident_sb = cpool.tile([P, P], F32, tag="ident_sb")
            nc.sync.dma_start(out=ident_sb[:], in_=ident_d[:])
            enc_nat_sb = cpool.tile([P, 8, H], BF16, tag="enc_nat_sb")
            nc.sync.dma_start(out=enc_nat_sb[:], in_=enc_nat_d[:])

            # ---------------- projections (bf16 matmuls) ----------------
            encproj_ps = []
            for co in range(HC):
                ep = pbig.tile([P, T_ENC], F32, tag=f"encproj{co}", name=f"encproj{co}")
                encproj_ps.append(ep)
                for half in range(2):
                    sl = slice(half * 512, (half + 1) * 512)
                    for ci in range(HC):
                        nc.tensor.matmul(
                            ep[:, sl],
                            W_sb[:, ci, co, :],
                            encT_sb[:, ci, sl],
                            start=(ci == 0),
                            stop=(ci == HC - 1),
                        )
            decproj_ps = paux.tile(
                [P, HC, DPC], F32, tag="aux", bufs=2, name="decproj_ps"
            )
            for co in range(HC):
                for ci in range(HC):
                    nc.tensor.matmul(
                        decproj_ps[:, co, :],
                        U_sb[:, ci, co, :],
                        decT_sb[:, ci, :],
                        start=(ci == 0),
                        stop=(ci == HC - 1),
                    )
            decproj_sb = cpool.tile([P, HC, DPC], F32, tag="decproj_sb")
            nc.vector.tensor_copy(decproj_sb[:], decproj_ps[:])

            # bf16 copy of enc_projT in SBUF for the 4x DVE pre-add
            # (per (c, half) so each cast can start as soon as its
            # accumulation chain completes)
            encproj_bf = cpool.tile([P, HC, T_ENC], BF16, tag="encproj_bf")
            for c in range(HC):
                for half in range(2):
                    sl = slice(half * 512, (half + 1) * 512)
                    nc.vector.tensor_copy(encproj_bf[:, c, sl], encproj_ps[c][:, sl])

            score_ps = pbig.tile([P, T_ENC], F32, tag="score", name="score")
            # scratch bank for HAM-warming filler matmuls: reuses the
            # encproj0 slot, which is dead once encproj_bf is built
            scratch_ps = pbig.tile([P, 512], F32, tag="encproj0", name="scratch")

            # ---------------- main loop ----------------
            d0 = 0
            for g, G in enumerate(GROUPS):
                th = wpool.tile(
                    [P, GMAX, HC, T_ENC], BF16, tag="th", bufs=4, name=f"th{g}"
                )
                for r in range(G):
                    d = d0 + r
                    for c in range(HC):
                        nc.vector.tensor_scalar_add(
                            th[:, r, c, :],
                            encproj_bf[:, c, :],
                            decproj_sb[:, c, d : d + 1],
                        )
                # in-place tanh over the whole group (engines stream
                # read-before-write, so src == dst is safe)
                nc.scalar.activation(th[:, :G, :, :], th[:, :G, :, :], TANH)
                n_fill = FILLERS[G] if g < len(GROUPS) - 2 else 0
                fill_per_r = (n_fill + G - 1) // G if n_fill else 0
                for r in range(G):
                    d = d0 + r
                    for c in range(HC):
                        for half in range(2):
                            sl = slice(half * 512, (half + 1) * 512)
                            nc.tensor.matmul(
                                score_ps[:, sl],
                                Vbig_sb[:, c, P - d : 2 * P - d],
                                th[:, r, c, sl],
                                start=(d == 0 and c == 0),
                                stop=(d == DPC - 1 and c == HC - 1),
                            )
                    for _ in range(fill_per_r):
                        nc.tensor.matmul(
                            scratch_ps[:],
                            Vbig_sb[:, 0, 0:P],
                            th[:, r, 0, 0:512],
                            start=True,
                            stop=True,
                            skip_group_check=True,
                        )
                d0 += G

            # ---------------- softmax (no max subtraction) ----------------
            expw = epool.tile([P, T_ENC], F32, tag="expw")
            sumexp = epool.tile([P, 1], F32, tag="sumexp")
            nc.scalar.activation(expw[:], score_ps[:], EXP, accum_out=sumexp[:])
            rec = epool.tile([P, 1], F32, tag="rec")
            nc.vector.reciprocal(rec[:], sumexp[:])
            wnorm = epool.tile([P, T_ENC], F32, tag="wnorm")
            nc.vector.tensor_scalar_mul(wnorm[:], expw[:], rec[:])
            nc.sync.dma_start(out=w_out_d[:], in_=wnorm[:])

            # ---------------- context = w @ enc (bf16 matmuls) ------------
            # transpose slots cycle over freed psum tags (aux x2 plus the
            # dead encproj1/scratch slots) for a 4-deep pipeline; each ctx
            # matmul is emitted right after its wT cast so PE interleaves
            # transposes with context accumulation
            wT_sb = epool.tile([P, 8, DPC], BF16, tag="wT_sb")
            ctx_ps = paux.tile([P, H], F32, tag="aux", bufs=2, name="ctx_ps")
            wt_tags = [("aux", paux, 2), ("encproj1", pbig, 1), ("encproj0", pbig, 1)]
            for t in range(8):
                tag, pool, bufs = wt_tags[t % 3]
                wT_ps = pool.tile([P, P], F32, tag=tag, bufs=bufs, name=f"wT{t}")
                nc.tensor.transpose(
                    wT_ps[:], wnorm[:, t * P : (t + 1) * P], ident_sb[:]
                )
                nc.vector.tensor_copy(wT_sb[:, t, :], wT_ps[:])
                nc.tensor.matmul(
                    ctx_ps[:],
                    wT_sb[:, t, :],
                    enc_nat_sb[:, t, :],
                    start=(t == 0),
                    stop=(t == 7),
                )
            ctx_sb = epool.tile([P, H], F32, tag="ctx_sb")
            nc.vector.tensor_copy(ctx_sb[:], ctx_ps[:])
            nc.sync.dma_start(out=ctx_out_d[:], in_=ctx_sb[:])

    nc.compile()
    return nc


def make_in_maps(encoder_outputs, decoder_outputs, W_a, U_a, V_a):
    bf = ml_dtypes.bfloat16
    enc = np.ascontiguousarray(np.asarray(encoder_outputs, dtype=np.float32))
    dec = np.ascontiguousarray(np.asarray(decoder_outputs, dtype=np.float32))
    W = np.asarray(W_a, dtype=np.float32)
    U = np.asarray(U_a, dtype=np.float32)
    V = np.asarray(V_a, dtype=np.float32).reshape(H)

    # partition-major host layouts (single DMA per tensor)
    enc_nat_all = np.ascontiguousarray(
        enc.reshape(B, 8, P, H).transpose(0, 2, 1, 3)
    ).astype(bf)  # [b, p, t, h]
    encT_all = np.ascontiguousarray(
        enc.transpose(0, 2, 1).reshape(B, HC, P, T_ENC).transpose(0, 2, 1, 3)
    ).astype(bf)  # [b, p, c, e]
    decT_full = dec.transpose(0, 2, 1).reshape(B, HC, P, T_DEC)  # [b, c, p, d]
    Wr = np.ascontiguousarray(
        W.reshape(HC, P, HC, P).transpose(1, 0, 2, 3)
    ).astype(bf)  # [p, ci, co, n]
    Ur = np.ascontiguousarray(U.reshape(HC, P, HC, P).transpose(1, 0, 2, 3)).astype(bf)

    Vbig = np.zeros((P, HC, 2 * P), dtype=bf)
    for c in range(HC):
        Vbig[:, c, P] = V[c * P : (c + 1) * P].astype(bf)
    ident = np.eye(P, dtype=np.float32)

    in_maps = []
    for core in range(N_CORES):
        b, half = core // 2, core % 2
        dlo = half * DPC
        decT_core = np.ascontiguousarray(
            decT_full[b][:, :, dlo : dlo + DPC].transpose(1, 0, 2)
        ).astype(bf)  # [p, c, d]
        in_maps.append(
            {
                "enc_nat": enc_nat_all[b],
                "encT": encT_all[b],
                "decT": decT_core,
                "W": Wr,
                "U": Ur,
                "Vbig": Vbig,
                "ident": ident,
            }
        )
    return in_maps


def kernel(encoder_outputs, decoder_outputs, W_a, U_a, V_a):
    from concourse.bass_utils import run_bass_kernel_spmd

    in_maps = make_in_maps(encoder_outputs, decoder_outputs, W_a, U_a, V_a)
    nc = build_graph()
    res = run_bass_kernel_spmd(nc, in_maps, core_ids=list(range(N_CORES)))

    ctx = np.zeros((B, T_DEC, H), dtype=np.float32)
    w = np.zeros((B, T_DEC, T_ENC), dtype=np.float32)
    for core in range(N_CORES):
        b, half = core // 2, core % 2
        dlo = half * DPC
        out = res.results[core]
        ctx[b, dlo : dlo + DPC] = out["ctx_out"]
        w[b, dlo : dlo + DPC] = out["w_out"]
    return ctx, w


# revision 16
# speedup vs baseline: 1.0387x; 1.0387x over previous
"""Bahdanau additive attention on 8 TRN2 NeuronCores.

Problem shapes: encoder [4, 1024, 256], decoder [4, 256, 256],
W_a/U_a [256, 256], V_a [256, 1].
reference:
    enc_proj = enc @ W_a                  [B, E, H]
    dec_proj = dec @ U_a                  [B, D, H]
    score[b,d,e] = sum_h V[h] * tanh(dec_proj[b,d,h] + enc_proj[b,e,h])
    w = softmax(score, axis=-1)           [B, D, E]
    ctx = w @ enc                         [B, D, H]
    return (ctx, w)

Sharding: 8 cores = (batch b = core//2) x (decoder-row half = core%2).
Each core owns 128 decoder rows of one batch element; outputs are
disjoint so no collectives are needed.

Per-core dataflow (h on SBUF partitions, 2 chunks of 128):
  - TensorE: bf16 projections enc_projT[h,e], dec_projT[h,d]
    (contract over h_in on partitions; host ships transposed layouts).
  - VectorE: bf16 tensor_scalar pre-add (4x mode) builds
    arg[h, (r,c,e)] = enc_projT[h,e] + dec_projT[h,d].
  - ScalarE (the bottleneck, ~222us of 1-elem/cycle/lane tanh): ONE
    in-place tanh ACTIVATE per group of 8 decoder rows (FD=16384)
    amortizes the fixed ~224-cycle cost; groups stream back-to-back.
  - TensorE: score rows via the shifted-V trick: lhsT is a 128-col
    slice of a [128, 256] tensor whose only nonzero column (index 128)
    holds V; slice [128-d, 256-d) puts V in column d, so PSUM row d
    accumulates score[d, :] while other rows get += 0.  Interleaved
    FILLER matmuls (into a scratch bank) keep TensorE's HAM activity
    window busy so the real matmuls run at the warm 2.4 GHz rate and
    the last group's straggler tail stays small.
  - softmax along the free axis (exp + accum_out; max subtraction is
    skipped: |score| <= sum|V| ~ 10, far from fp32 overflow),
    TensorE transposes of w, bf16 context matmul, DMA out.
"""

import os
import sys

for _p in (
    "/opt/trn_rl_repo",
    "/root/.axon_site",
    "/root/.axon_site/_ro/trn_rl_repo",
    "/root/.axon_site/_ro/pypackages",
):
    if os.path.isdir(_p) and _p not in sys.path:
        sys.path.append(_p)

import ml_dtypes
import numpy as np

import concourse.mybir as mybir
from concourse import bacc, bass, tile

F32 = mybir.dt.float32
BF16 = mybir.dt.bfloat16

B, T_ENC, T_DEC, H = 4, 1024, 256, 256
P = 128  # SBUF partitions
HC = H // P  # h chunks (2)
DPC = 128  # decoder rows per core
N_CORES = 8

GROUPS = [4] + [8] * 15 + [4]
assert sum(GROUPS) == DPC
GMAX = max(GROUPS)
# filler matmuls per group: pad PE work to ~the ACT group duration
# ACT group ~= (224 + G*2048)/1.2 ns; warm matmul ~= 218 ns (N=512)
FILLERS = {4: 16, 8: 32}

MODE = os.environ.get("ATTN_KERNEL_MODE", "v4")


def build_graph(mode=MODE):
    nc = bacc.Bacc("TRN2", target_bir_lowering=False, debug=False)

    # partition-major host layouts: one clean DMA per tensor
    enc_nat_d = nc.declare_dram_parameter("enc_nat", [P, 8, H], BF16, isOutput=False)
    encT_d = nc.declare_dram_parameter("encT", [P, HC, T_ENC], BF16, isOutput=False)
    decT_d = nc.declare_dram_parameter("decT", [P, HC, DPC], BF16, isOutput=False)
    W_d = nc.declare_dram_parameter("W", [P, HC, HC, P], BF16, isOutput=False)
    U_d = nc.declare_dram_parameter("U", [P, HC, HC, P], BF16, isOutput=False)
    Vbig_d = nc.declare_dram_parameter("Vbig", [P, HC, 2 * P], BF16, isOutput=False)
    ident_d = nc.declare_dram_parameter("ident", [P, P], F32, isOutput=False)
    w_out_d = nc.declare_dram_parameter("w_out", [DPC, T_ENC], F32, isOutput=True)
    ctx_out_d = nc.declare_dram_parameter("ctx_out", [DPC, H], F32, isOutput=True)

    TANH = mybir.ActivationFunctionType.Tanh
    EXP = mybir.ActivationFunctionType.Exp

    with tile.TileContext(nc) as tc:
        with (
            tc.tile_pool(name="const", bufs=1) as cpool,
            tc.tile_pool(name="psum_big", bufs=1, space="PSUM") as pbig,
            tc.tile_pool(name="psum_aux", bufs=1, space="PSUM") as paux,
            tc.tile_pool(name="work", bufs=1) as wpool,
            tc.tile_pool(name="epi", bufs=1) as epool,
        ):
            # ---------------- constants in (one DMA each) ----------------
            encT_sb = cpool.tile([P, HC, T_ENC], BF16, tag="encT_sb")
            nc.sync.dma_start(out=encT_sb[:], in_=encT_d[:])
            W_sb = cpool.tile([P, HC, HC, P], BF16, tag="W_sb")
            nc.sync.dma_start(out=W_sb[:], in_=W_d[:])
            U_sb = cpool.tile([P, HC, HC, P], BF16, tag="U_sb")
            nc.sync.dma_start(out=U_sb[:], in_=U_d[:])
            decT_sb = cpool.tile([P, HC, DPC], BF16, tag="decT_sb")
            nc.sync.dma_start(out=decT_sb[:], in_=decT_d[:])
            Vbig_sb = cpool.tile([P, HC, 2 * P], BF16, tag="Vbig_sb")
            nc.sync.dma_start(out=Vbig_sb[:], in_=Vbig_d[:])
            ident_sb = cpool.tile([P, P], F32, tag="ident_sb")
            nc.sync.dma_start(out=ident_sb[:], in_=ident_d[:])
            enc_nat_sb = cpool.tile([P, 8, H], BF16, tag="enc_nat_sb")
            nc.sync.dma_start(out=enc_nat_sb[:], in_=enc_nat_d[:])

            # ---------------- projections (bf16 matmuls) ----------------
            encproj_ps = []
            for co in range(HC):
                ep = pbig.tile([P, T_ENC], F32, tag=f"encproj{co}", name=f"encproj{co}")
                encproj_ps.append(ep)
                for half in range(2):
                    sl = slice(half * 512, (half + 1) * 512)
                    for ci in range(HC):
                        nc.tensor.matmul(
                            ep[:, sl],
                            W_sb[:, ci, co, :],
                            encT_sb[:, ci, sl],
                            start=(ci == 0),
                            stop=(ci == HC - 1),
                        )
            decproj_ps = paux.tile(
                [P, HC, DPC], F32, tag="aux", bufs=2, name="decproj_ps"
            )
            for co in range(HC):
                for ci in range(HC):
                    nc.tensor.matmul(
                        decproj_ps[:, co, :],
                        U_sb[:, ci, co, :],
                        decT_sb[:, ci, :],
                        start=(ci == 0),
                        stop=(ci == HC - 1),
                    )
            decproj_sb = cpool.tile([P, HC, DPC], F32, tag="decproj_sb")
            nc.vector.tensor_copy(decproj_sb[:], decproj_ps[:])

            # bf16 copy of enc_projT in SBUF for the 4x DVE pre-add
            encproj_bf = cpool.tile([P, HC, T_ENC], BF16, tag="encproj_bf")
            for c in range(HC):
                nc.vector.tensor_copy(encproj_bf[:, c, :], encproj_ps[c][:])

            score_ps = pbig.tile([P, T_ENC], F32, tag="score", name="score")
            # scratch bank for HAM-warming filler matmuls: reuses the
            # encproj0 slot, which is dead once encproj_bf is built
            scratch_ps = pbig.tile([P, 512], F32, tag="encproj0", name="scratch")

            # ---------------- main loop ----------------
            d0 = 0
            for g, G in enumerate(GROUPS):
                th = wpool.tile(
                    [P, GMAX, HC, T_ENC], BF16, tag="th", bufs=3, name=f"th{g}"
                )
                for r in range(G):
                    d = d0 + r
                    for c in range(HC):
                        nc.vector.tensor_scalar_add(
                            th[:, r, c, :],
                            encproj_bf[:, c, :],
                            decproj_sb[:, c, d : d + 1],
                        )
                # in-place tanh over the whole group (engines stream
                # read-before-write, so src == dst is safe)
                nc.scalar.activation(th[:, :G, :, :], th[:, :G, :, :], TANH)
                n_fill = FILLERS[G] if g < len(GROUPS) - 2 else 0
                fill_per_r = (n_fill + G - 1) // G if n_fill else 0
                for r in range(G):
                    d = d0 + r
                    for c in range(HC):
                        for half in range(2):
                            sl = slice(half * 512, (half + 1) * 512)
                            nc.tensor.matmul(
                                score_ps[:, sl],
                                Vbig_sb[:, c, P - d : 2 * P - d],
                                th[:, r, c, sl],
                                start=(d == 0 and c == 0),
                                stop=(d == DPC - 1 and c == HC - 1),
                            )
                    for _ in range(fill_per_r):
                        nc.tensor.matmul(
                            scratch_ps[:],
                            Vbig_sb[:, 0, 0:P],
                            th[:, r, 0, 0:512],
                            start=True,
                            stop=True,
                            skip_group_check=True,
                        )
                d0 += G

            # ---------------- softmax (no max subtraction) ----------------
            expw = epool.tile([P, T_ENC], F32, tag="expw")
            sumexp = epool.tile([P, 1], F32, tag="sumexp")
            nc.scalar.activation(expw[:], score_ps[:], EXP, accum_out=sumexp[:])
            rec = epool.tile([P, 1], F32, tag="rec")
            nc.vector.reciprocal(rec[:], sumexp[:])
            wnorm = epool.tile([P, T_ENC], F32, tag="wnorm")
            nc.vector.tensor_scalar_mul(wnorm[:], expw[:], rec[:])
            nc.sync.dma_start(out=w_out_d[:], in_=wnorm[:])

            # ---------------- context = w @ enc (bf16 matmuls) ------------
            # transpose slots cycle over freed psum tags (aux x2 plus the
            # dead encproj1/scratch slots) for a 4-deep pipeline; each ctx
            # matmul is emitted right after its wT cast so PE interleaves
            # transposes with context accumulation
            wT_sb = epool.tile([P, 8, DPC], BF16, tag="wT_sb")
            for t in range(8):
                wT_ps = paux.tile([P, P], F32, tag="aux", bufs=2, name=f"wT{t}")
                nc.tensor.transpose(
                    wT_ps[:], wnorm[:, t * P : (t + 1) * P], ident_sb[:]
                )
                nc.vector.tensor_copy(wT_sb[:, t, :], wT_ps[:])
            ctx_ps = paux.tile([P, H], F32, tag="aux", bufs=2, name="ctx_ps")
            for t in range(8):
                nc.tensor.matmul(
                    ctx_ps[:],
                    wT_sb[:, t, :],
                    enc_nat_sb[:, t, :],
                    start=(t == 0),
                    stop=(t == 7),
                )
            ctx_sb = epool.tile([P, H], F32, tag="ctx_sb")
            nc.vector.tensor_copy(ctx_sb[:], ctx_ps[:])
            nc.sync.dma_start(out=ctx_out_d[:], in_=ctx_sb[:])

    nc.compile()
    return nc


def make_in_maps(encoder_outputs, decoder_outputs, W_a, U_a, V_a):
    bf = ml_dtypes.bfloat16
    enc = np.ascontiguousarray(np.asarray(encoder_outputs, dtype=np.float32))
    dec = np.ascontiguousarray(np.asarray(decoder_outputs, dtype=np.float32))
    W = np.asarray(W_a, dtype=np.float32)
    U = np.asarray(U_a, dtype=np.float32)
    V = np.asarray(V_a, dtype=np.float32).reshape(H)

    # partition-major host layouts (single DMA per tensor)
    enc_nat_all = np.ascontiguousarray(
        enc.reshape(B, 8, P, H).transpose(0, 2, 1, 3)
    ).astype(bf)  # [b, p, t, h]
    encT_all = np.ascontiguousarray(
        enc.transpose(0, 2, 1).reshape(B, HC, P, T_ENC).transpose(0, 2, 1, 3)
    ).astype(bf)  # [b, p, c, e]
    decT_full = dec.transpose(0, 2, 1).reshape(B, HC, P, T_DEC)  # [b, c, p, d]
    Wr = np.ascontiguousarray(
        W.reshape(HC, P, HC, P).transpose(1, 0, 2, 3)
    ).astype(bf)  # [p, ci, co, n]
    Ur = np.ascontiguousarray(U.reshape(HC, P, HC, P).transpose(1, 0, 2, 3)).astype(bf)

    Vbig = np.zeros((P, HC, 2 * P), dtype=bf)
    for c in range(HC):
        Vbig[:, c, P] = V[c * P : (c + 1) * P].astype(bf)
    ident = np.eye(P, dtype=np.float32)

    in_maps = []
    for core in range(N_CORES):
        b, half = core // 2, core % 2
        dlo = half * DPC
        decT_core = np.ascontiguousarray(
            decT_full[b][:, :, dlo : dlo + DPC].transpose(1, 0, 2)
        ).astype(bf)  # [p, c, d]
        in_maps.append(
            {
                "enc_nat": enc_nat_all[b],
                "encT": encT_all[b],
                "decT": decT_core,
                "W": Wr,
                "U": Ur,
                "Vbig": Vbig,
                "ident": ident,
            }
        )
    return in_maps


def kernel(encoder_outputs, decoder_outputs, W_a, U_a, V_a):
    from concourse.bass_utils import run_bass_kernel_spmd

    in_maps = make_in_maps(encoder_outputs, decoder_outputs, W_a, U_a, V_a)
    nc = build_graph()
    res = run_bass_kernel_spmd(nc, in_maps, core_ids=list(range(N_CORES)))

    ctx = np.zeros((B, T_DEC, H), dtype=np.float32)
    w = np.zeros((B, T_DEC, T_ENC), dtype=np.float32)
    for core in range(N_CORES):
        b, half = core // 2, core % 2
        dlo = half * DPC
        out = res.results[core]
        ctx[b, dlo : dlo + DPC] = out["ctx_out"]
        w[b, dlo : dlo + DPC] = out["w_out"]
    return ctx, w


# revision 17
# speedup vs baseline: 2.9168x; 2.8083x over previous
"""Bahdanau additive attention on 8 TRN2 NeuronCores — sine-expansion kernel.

reference:
    enc_proj = enc @ W_a   (args b),  dec_proj = dec @ U_a   (args a)
    score[b,d,e] = sum_h V[h] * tanh(a[d,h] + b[e,h])
    w = softmax(score, -1); ctx = w @ enc; return (ctx, w)

Key idea: on the bounded arg range, tanh(u) ~= sum_{k=1..K} c_k sin(k w1 u)
(least-squares fit, K=12, max err ~1e-3 over [-U0, U0] with U0 a hard
bound from the actual inputs).  Each harmonic is EXACTLY rank-2:
sin(kw(a+b)) = sin(kwa)cos(kwb) + cos(kwa)sin(kwb), so the score reduces
to 8K TensorE matmuls over h instead of 268M ScalarE tanh evaluations.

ScalarE's Sin only accepts [-pi, pi], so only the base frequency is
evaluated there: theta = w1*proj with |theta| <= pi*max|proj|/UP and the
host picks UP = max(1.35*U0, 2.2*max|proj|) so even cos's +pi/2 bias
stays in range.  Harmonics k>=3 come from the Chebyshev recurrence on
VectorE (bf16 2x mode):  x_k = 2cos(theta)*x_{k-1} - x_{k-2}  — the
recurrence is marginally stable (solutions are the bounded cos/sin
themselves) so bf16 error grows only ~linearly in k.  The V*c_k folding
runs on the now-idle ScalarE as per-partition-scaled Copies.

Sharding: 8 cores = (batch b = core//2) x (128 decoder rows, core%2);
outputs disjoint, no collectives.  Softmax skips max-subtraction
(|score| <= sum|V| ~ 10, far from fp32 overflow).
"""

import math
import os
import sys

for _p in (
    "/opt/trn_rl_repo",
    "/root/.axon_site",
    "/root/.axon_site/_ro/trn_rl_repo",
    "/root/.axon_site/_ro/pypackages",
):
    if os.path.isdir(_p) and _p not in sys.path:
        sys.path.append(_p)

import ml_dtypes
import numpy as np

import concourse.mybir as mybir
from concourse import bacc, bass, tile

F32 = mybir.dt.float32
BF16 = mybir.dt.bfloat16

B, T_ENC, T_DEC, H = 4, 1024, 256, 256
P = 128
HC = H // P  # 2 chunks of h
DPC = 128
N_CORES = 8

K_HARM = 12

MODE = os.environ.get("ATTN_KERNEL_MODE", "v7")


def fit_sine(u0, up):
    om = np.arange(1, K_HARM + 1) * math.pi / up
    u = np.linspace(-u0, u0, 8001)
    A = np.sin(np.outer(u, om))
    c, *_ = np.linalg.lstsq(A, np.tanh(u), rcond=None)
    return om, c, float(np.abs(A @ c - np.tanh(u)).max())


def build_graph(omega1, mode=MODE):
    nc = bacc.Bacc("TRN2", target_bir_lowering=False, debug=False)

    enc_nat_d = nc.declare_dram_parameter("enc_nat", [P, 8, H], BF16, isOutput=False)
    encT_d = nc.declare_dram_parameter("encT", [P, HC, T_ENC], BF16, isOutput=False)
    decT_d = nc.declare_dram_parameter("decT", [P, HC, DPC], BF16, isOutput=False)
    W_d = nc.declare_dram_parameter("W", [P, HC, HC, P], BF16, isOutput=False)
    U_d = nc.declare_dram_parameter("U", [P, HC, HC, P], BF16, isOutput=False)
    Vc_d = nc.declare_dram_parameter("Vc", [P, HC, K_HARM], F32, isOutput=False)
    ident_d = nc.declare_dram_parameter("ident", [P, P], F32, isOutput=False)
    w_out_d = nc.declare_dram_parameter("w_out", [DPC, T_ENC], F32, isOutput=True)
    ctx_out_d = nc.declare_dram_parameter("ctx_out", [DPC, H], F32, isOutput=True)

    SIN = mybir.ActivationFunctionType.Sin
    EXP = mybir.ActivationFunctionType.Exp
    COPY = mybir.ActivationFunctionType.Copy
    MULT = mybir.AluOpType.mult
    SUB = mybir.AluOpType.subtract
    ADD = mybir.AluOpType.add

    with tile.TileContext(nc) as tc:
        with (
            tc.tile_pool(name="const", bufs=1) as cpool,
            tc.tile_pool(name="psum_big", bufs=1, space="PSUM") as pbig,
            tc.tile_pool(name="psum_aux", bufs=1, space="PSUM") as paux,
            tc.tile_pool(name="trig", bufs=1) as tpool,
            tc.tile_pool(name="epi", bufs=1) as epool,
        ):
            # ---------------- constants in ----------------
            encT_sb = cpool.tile([P, HC, T_ENC], BF16, tag="encT_sb")
            nc.sync.dma_start(out=encT_sb[:], in_=encT_d[:])
            W_sb = cpool.tile([P, HC, HC, P], BF16, tag="W_sb")
            nc.sync.dma_start(out=W_sb[:], in_=W_d[:])
            U_sb = cpool.tile([P, HC, HC, P], BF16, tag="U_sb")
            nc.sync.dma_start(out=U_sb[:], in_=U_d[:])
            decT_sb = cpool.tile([P, HC, DPC], BF16, tag="decT_sb")
            nc.sync.dma_start(out=decT_sb[:], in_=decT_d[:])
            Vc_sb = cpool.tile([P, HC, K_HARM], F32, tag="Vc_sb")
            nc.sync.dma_start(out=Vc_sb[:], in_=Vc_d[:])
            ident_sb = cpool.tile([P, P], F32, tag="ident_sb")
            nc.sync.dma_start(out=ident_sb[:], in_=ident_d[:])
            enc_nat_sb = cpool.tile([P, 8, H], BF16, tag="enc_nat_sb")
            nc.sync.dma_start(out=enc_nat_sb[:], in_=enc_nat_d[:])

            halfpi = cpool.tile([P, 1], F32, tag="halfpi")
            nc.vector.memset(halfpi[:], math.pi / 2.0)

            # ---------------- projections (bf16, into PSUM) ----------------
            encproj_ps = pbig.tile([P, HC, T_ENC], F32, tag="encproj", name="encproj")
            for co in range(HC):
                for half in range(2):
                    sl = slice(half * 512, (half + 1) * 512)
                    for ci in range(HC):
                        nc.tensor.matmul(
                            encproj_ps[:, co, sl],
                            W_sb[:, ci, co, :],
                            encT_sb[:, ci, sl],
                            start=(ci == 0),
                            stop=(ci == HC - 1),
                        )
            decproj_ps = paux.tile(
                [P, HC, DPC], F32, tag="aux", bufs=2, name="decproj_ps"
            )
            for co in range(HC):
                for ci in range(HC):
                    nc.tensor.matmul(
                        decproj_ps[:, co, :],
                        U_sb[:, ci, co, :],
                        decT_sb[:, ci, :],
                        start=(ci == 0),
                        stop=(ci == HC - 1),
                    )

            score_ps = pbig.tile([P, T_ENC], F32, tag="score", name="score")

            # ---------------- base-frequency trig on ScalarE ----------------
            def trig_tiles(src_ps, fd, side):
                s1 = tpool.tile([P, HC, fd], BF16, tag=f"s{side}", bufs=5, name=f"s1{side}")
                c1 = tpool.tile([P, HC, fd], BF16, tag=f"c{side}", bufs=5, name=f"c1{side}")
                nc.scalar.activation(s1[:], src_ps[:], SIN, scale=omega1)
                nc.scalar.activation(c1[:], src_ps[:], SIN, scale=omega1, bias=halfpi[:])
                d1 = tpool.tile([P, HC, fd], BF16, tag=f"d{side}", bufs=1, name=f"d1{side}")
                nc.vector.tensor_scalar_mul(d1[:], c1[:], 2.0)
                s2 = tpool.tile([P, HC, fd], BF16, tag=f"s{side}", bufs=5, name=f"s2{side}")
                nc.vector.tensor_tensor(s2[:], d1[:], s1[:], MULT)
                c2 = tpool.tile([P, HC, fd], BF16, tag=f"c{side}", bufs=5, name=f"c2{side}")
                nc.vector.tensor_tensor(c2[:], s1[:], s1[:], MULT)
                nc.vector.tensor_scalar(c2[:], c2[:], -2.0, 1.0, MULT, ADD)
                return {1: s1, 2: s2}, {1: c1, 2: c2}, d1

            se, ce, d1e = trig_tiles(encproj_ps, T_ENC, "e")
            sd, cd, d1d = trig_tiles(decproj_ps, DPC, "d")

            def advance(k, s, c, d1, fd, side):
                sk = tpool.tile([P, HC, fd], BF16, tag=f"s{side}", bufs=5, name=f"s{k}{side}")
                nc.vector.tensor_tensor(sk[:], d1[:], s[k - 1][:], MULT)
                nc.vector.tensor_tensor(sk[:], sk[:], s[k - 2][:], SUB)
                ck = tpool.tile([P, HC, fd], BF16, tag=f"c{side}", bufs=5, name=f"c{k}{side}")
                nc.vector.tensor_tensor(ck[:], d1[:], c[k - 1][:], MULT)
                nc.vector.tensor_tensor(ck[:], ck[:], c[k - 2][:], SUB)
                s[k] = sk
                c[k] = ck

            # ---------------- harmonic loop ----------------
            n_mm = 0
            for k in range(1, K_HARM + 1):
                if k >= 3:
                    advance(k, se, ce, d1e, T_ENC, "e")
                    advance(k, sd, cd, d1d, DPC, "d")
                # fold V*c_k into the d-side on ScalarE (idle engine):
                # per-chunk per-partition scale via activation Copy
                fs = tpool.tile([P, HC, DPC], BF16, tag="fs", bufs=4, name=f"fs{k}")
                fc = tpool.tile([P, HC, DPC], BF16, tag="fc", bufs=4, name=f"fc{k}")
                for c in range(HC):
                    nc.scalar.activation(
                        fs[:, c, :], sd[k][:, c, :], COPY, scale=Vc_sb[:, c, k - 1 : k]
                    )
                    nc.scalar.activation(
                        fc[:, c, :], cd[k][:, c, :], COPY, scale=Vc_sb[:, c, k - 1 : k]
                    )
                last = k == K_HARM
                for F, G in ((fs, ce[k]), (fc, se[k])):
                    for c in range(HC):
                        for half in range(2):
                            sl = slice(half * 512, (half + 1) * 512)
                            nc.tensor.matmul(
                                score_ps[:, sl],
                                F[:, c, :],
                                G[:, c, sl],
                                start=(n_mm == 0),
                                stop=(last and G is se[k] and c == HC - 1),
                            )
                        n_mm += 2
                # drop references no longer needed by the recurrence
                if k >= 3:
                    del se[k - 2], ce[k - 2], sd[k - 2], cd[k - 2]

            # ---------------- softmax (no max subtraction) ----------------
            expw = epool.tile([P, T_ENC], F32, tag="expw")
            sumexp = epool.tile([P, 1], F32, tag="sumexp")
            nc.scalar.activation(expw[:], score_ps[:], EXP, accum_out=sumexp[:])
            rec = epool.tile([P, 1], F32, tag="rec")
            nc.vector.reciprocal(rec[:], sumexp[:])
            wnorm = epool.tile([P, T_ENC], F32, tag="wnorm")
            nc.vector.tensor_scalar_mul(wnorm[:], expw[:], rec[:])
            nc.sync.dma_start(out=w_out_d[:], in_=wnorm[:])

            # ---------------- context = w @ enc (bf16) ----------------
            wT_sb = epool.tile([P, 8, DPC], BF16, tag="wT_sb")
            for t in range(8):
                wT_ps = paux.tile([P, P], F32, tag="aux", bufs=2, name=f"wT{t}")
                nc.tensor.transpose(
                    wT_ps[:], wnorm[:, t * P : (t + 1) * P], ident_sb[:]
                )
                nc.vector.tensor_copy(wT_sb[:, t, :], wT_ps[:])
            ctx_ps = paux.tile([P, H], F32, tag="aux", bufs=2, name="ctx_ps")
            for t in range(8):
                nc.tensor.matmul(
                    ctx_ps[:],
                    wT_sb[:, t, :],
                    enc_nat_sb[:, t, :],
                    start=(t == 0),
                    stop=(t == 7),
                )
            ctx_sb = epool.tile([P, H], F32, tag="ctx_sb")
            nc.vector.tensor_copy(ctx_sb[:], ctx_ps[:])
            nc.sync.dma_start(out=ctx_out_d[:], in_=ctx_sb[:])

    nc.compile()
    return nc


def make_in_maps(encoder_outputs, decoder_outputs, W_a, U_a, V_a):
    bf = ml_dtypes.bfloat16
    enc = np.ascontiguousarray(np.asarray(encoder_outputs, dtype=np.float32))
    dec = np.ascontiguousarray(np.asarray(decoder_outputs, dtype=np.float32))
    W = np.asarray(W_a, dtype=np.float32)
    U = np.asarray(U_a, dtype=np.float32)
    V = np.asarray(V_a, dtype=np.float32).reshape(H)

    amax = float(np.abs(dec @ U).max())
    bmax = float(np.abs(enc @ W).max())
    u0 = (amax + bmax) * 1.001
    # period: fit quality wants ~1.35*u0; Sin range safety (theta + pi/2
    # <= pi even for the cos bias) wants >= 2.2*max-side
    up = max(1.35 * u0, 2.2 * max(amax, bmax))
    omegas, coefs, fit_err = fit_sine(u0, up)

    enc_nat_all = np.ascontiguousarray(
        enc.reshape(B, 8, P, H).transpose(0, 2, 1, 3)
    ).astype(bf)
    encT_all = np.ascontiguousarray(
        enc.transpose(0, 2, 1).reshape(B, HC, P, T_ENC).transpose(0, 2, 1, 3)
    ).astype(bf)
    decT_full = dec.transpose(0, 2, 1).reshape(B, HC, P, T_DEC)
    Wr = np.ascontiguousarray(W.reshape(HC, P, HC, P).transpose(1, 0, 2, 3)).astype(bf)
    Ur = np.ascontiguousarray(U.reshape(HC, P, HC, P).transpose(1, 0, 2, 3)).astype(bf)

    Vc = np.zeros((P, HC, K_HARM), dtype=np.float32)
    for c in range(HC):
        Vc[:, c, :] = V[c * P : (c + 1) * P, None] * coefs[None, :]
    ident = np.eye(P, dtype=np.float32)

    in_maps = []
    for core in range(N_CORES):
        b, half = core // 2, core % 2
        dlo = half * DPC
        decT_core = np.ascontiguousarray(
            decT_full[b][:, :, dlo : dlo + DPC].transpose(1, 0, 2)
        ).astype(bf)
        in_maps.append(
            {
                "enc_nat": enc_nat_all[b],
                "encT": encT_all[b],
                "decT": decT_core,
                "W": Wr,
                "U": Ur,
                "Vc": Vc,
                "ident": ident,
            }
        )
    return in_maps, float(omegas[0])


def kernel(encoder_outputs, decoder_outputs, W_a, U_a, V_a):
    from concourse.bass_utils import run_bass_kernel_spmd

    in_maps, omega1 = make_in_maps(encoder_outputs, decoder_outputs, W_a, U_a, V_a)
    nc = build_graph(omega1)
    res = run_bass_kernel_spmd(nc, in_maps, core_ids=list(range(N_CORES)))

    ctx = np.zeros((B, T_DEC, H), dtype=np.float32)
    w = np.zeros((B, T_DEC, T_ENC), dtype=np.float32)
    for core in range(N_CORES):
        b, half = core // 2, core % 2
        dlo = half * DPC
        out = res.results[core]
        ctx[b, dlo : dlo + DPC] = out["ctx_out"]
        w[b, dlo : dlo + DPC] = out["w_out"]
    return ctx, w


# revision 19
# speedup vs baseline: 2.9259x; 1.0031x over previous
"""Bahdanau additive attention on 8 TRN2 NeuronCores — sine-expansion kernel.

reference:
    enc_proj = enc @ W_a   (args b),  dec_proj = dec @ U_a   (args a)
    score[b,d,e] = sum_h V[h] * tanh(a[d,h] + b[e,h])
    w = softmax(score, -1); ctx = w @ enc; return (ctx, w)

Key idea: on the bounded arg range, tanh(u) ~= sum_{k=1..K} c_k sin(k w1 u)
(least-squares fit, K=12, max err ~1e-3 over [-U0, U0] with U0 a hard
bound from the actual inputs).  Each harmonic is EXACTLY rank-2:
sin(kw(a+b)) = sin(kwa)cos(kwb) + cos(kwa)sin(kwb), so the score reduces
to 8K TensorE matmuls over h instead of 268M ScalarE tanh evaluations.

ScalarE's Sin only accepts [-pi, pi], so only the base frequency is
evaluated there: theta = w1*proj with |theta| <= pi*max|proj|/UP and the
host picks UP = max(1.35*U0, 2.2*max|proj|) so even cos's +pi/2 bias
stays in range.  Harmonics k>=3 come from the Chebyshev recurrence on
VectorE (bf16 2x mode):  x_k = 2cos(theta)*x_{k-1} - x_{k-2}  — the
recurrence is marginally stable (solutions are the bounded cos/sin
themselves) so bf16 error grows only ~linearly in k.  The V*c_k folding
runs on the now-idle ScalarE as per-partition-scaled Copies.

Sharding: 8 cores = (batch b = core//2) x (128 decoder rows, core%2);
outputs disjoint, no collectives.  Softmax skips max-subtraction
(|score| <= sum|V| ~ 10, far from fp32 overflow).
"""

import math
import os
import sys

for _p in (
    "/opt/trn_rl_repo",
    "/root/.axon_site",
    "/root/.axon_site/_ro/trn_rl_repo",
    "/root/.axon_site/_ro/pypackages",
):
    if os.path.isdir(_p) and _p not in sys.path:
        sys.path.append(_p)

import ml_dtypes
import numpy as np

import concourse.mybir as mybir
from concourse import bacc, bass, tile

F32 = mybir.dt.float32
BF16 = mybir.dt.bfloat16

B, T_ENC, T_DEC, H = 4, 1024, 256, 256
P = 128
HC = H // P  # 2 chunks of h
DPC = 128
N_CORES = 8

K_HARM = 12

MODE = os.environ.get("ATTN_KERNEL_MODE", "v7")


def fit_sine(u0, up):
    om = np.arange(1, K_HARM + 1) * math.pi / up
    u = np.linspace(-u0, u0, 8001)
    A = np.sin(np.outer(u, om))
    c, *_ = np.linalg.lstsq(A, np.tanh(u), rcond=None)
    return om, c, float(np.abs(A @ c - np.tanh(u)).max())


def build_graph(omega1, mode=MODE):
    nc = bacc.Bacc("TRN2", target_bir_lowering=False, debug=False)

    enc_nat_d = nc.declare_dram_parameter("enc_nat", [P, 8, H], BF16, isOutput=False)
    encT_d = nc.declare_dram_parameter("encT", [P, HC, T_ENC], BF16, isOutput=False)
    decT_d = nc.declare_dram_parameter("decT", [P, HC, DPC], BF16, isOutput=False)
    W_d = nc.declare_dram_parameter("W", [P, HC, HC, P], BF16, isOutput=False)
    U_d = nc.declare_dram_parameter("U", [P, HC, HC, P], BF16, isOutput=False)
    Vc_d = nc.declare_dram_parameter("Vc", [P, HC, K_HARM], F32, isOutput=False)
    ident_d = nc.declare_dram_parameter("ident", [P, P], F32, isOutput=False)
    w_out_d = nc.declare_dram_parameter("w_out", [DPC, T_ENC], F32, isOutput=True)
    ctx_out_d = nc.declare_dram_parameter("ctx_out", [DPC, H], F32, isOutput=True)

    SIN = mybir.ActivationFunctionType.Sin
    EXP = mybir.ActivationFunctionType.Exp
    COPY = mybir.ActivationFunctionType.Copy
    MULT = mybir.AluOpType.mult
    SUB = mybir.AluOpType.subtract
    ADD = mybir.AluOpType.add

    with tile.TileContext(nc) as tc:
        with (
            tc.tile_pool(name="const", bufs=1) as cpool,
            tc.tile_pool(name="psum_big", bufs=1, space="PSUM") as pbig,
            tc.tile_pool(name="psum_aux", bufs=1, space="PSUM") as paux,
            tc.tile_pool(name="trig", bufs=1) as tpool,
            tc.tile_pool(name="epi", bufs=1) as epool,
        ):
            # ---------------- constants in ----------------
            encT_sb = cpool.tile([P, HC, T_ENC], BF16, tag="encT_sb")
            nc.sync.dma_start(out=encT_sb[:], in_=encT_d[:])
            W_sb = cpool.tile([P, HC, HC, P], BF16, tag="W_sb")
            nc.sync.dma_start(out=W_sb[:], in_=W_d[:])
            U_sb = cpool.tile([P, HC, HC, P], BF16, tag="U_sb")
            nc.sync.dma_start(out=U_sb[:], in_=U_d[:])
            decT_sb = cpool.tile([P, HC, DPC], BF16, tag="decT_sb")
            nc.sync.dma_start(out=decT_sb[:], in_=decT_d[:])
            Vc_sb = cpool.tile([P, HC, K_HARM], F32, tag="Vc_sb")
            nc.sync.dma_start(out=Vc_sb[:], in_=Vc_d[:])
            ident_sb = cpool.tile([P, P], F32, tag="ident_sb")
            nc.sync.dma_start(out=ident_sb[:], in_=ident_d[:])
            enc_nat_sb = cpool.tile([P, 8, H], BF16, tag="enc_nat_sb")
            nc.sync.dma_start(out=enc_nat_sb[:], in_=enc_nat_d[:])

            halfpi = cpool.tile([P, 1], F32, tag="halfpi")
            nc.vector.memset(halfpi[:], math.pi / 2.0)

            # ---------------- projections (bf16, into PSUM) ----------------
            encproj_ps = pbig.tile([P, HC, T_ENC], F32, tag="encproj", name="encproj")
            for co in range(HC):
                for half in range(2):
                    sl = slice(half * 512, (half + 1) * 512)
                    for ci in range(HC):
                        nc.tensor.matmul(
                            encproj_ps[:, co, sl],
                            W_sb[:, ci, co, :],
                            encT_sb[:, ci, sl],
                            start=(ci == 0),
                            stop=(ci == HC - 1),
                        )
            decproj_ps = paux.tile(
                [P, HC, DPC], F32, tag="aux", bufs=2, name="decproj_ps"
            )
            for co in range(HC):
                for ci in range(HC):
                    nc.tensor.matmul(
                        decproj_ps[:, co, :],
                        U_sb[:, ci, co, :],
                        decT_sb[:, ci, :],
                        start=(ci == 0),
                        stop=(ci == HC - 1),
                    )

            score_ps = pbig.tile([P, T_ENC], F32, tag="score", name="score")

            # ---------------- base-frequency trig on ScalarE ----------------
            def trig_tiles(src_ps, fd, side):
                # cos first: the DVE chain starts from d1 = 2*cos while
                # ScalarE still evaluates sin
                s1 = tpool.tile([P, HC, fd], BF16, tag=f"s{side}", bufs=5, name=f"s1{side}")
                c1 = tpool.tile([P, HC, fd], BF16, tag=f"c{side}", bufs=5, name=f"c1{side}")
                nc.scalar.activation(c1[:], src_ps[:], SIN, scale=omega1, bias=halfpi[:])
                nc.scalar.activation(s1[:], src_ps[:], SIN, scale=omega1)
                d1 = tpool.tile([P, HC, fd], BF16, tag=f"d{side}", bufs=1, name=f"d1{side}")
                nc.vector.tensor_scalar_mul(d1[:], c1[:], 2.0)
                s2 = tpool.tile([P, HC, fd], BF16, tag=f"s{side}", bufs=5, name=f"s2{side}")
                nc.vector.tensor_tensor(s2[:], d1[:], s1[:], MULT)
                c2 = tpool.tile([P, HC, fd], BF16, tag=f"c{side}", bufs=5, name=f"c2{side}")
                nc.vector.tensor_tensor(c2[:], s1[:], s1[:], MULT)
                nc.vector.tensor_scalar(c2[:], c2[:], -2.0, 1.0, MULT, ADD)
                return {1: s1, 2: s2}, {1: c1, 2: c2}, d1

            # d-side first: its projection finishes earlier, so DVE gets
            # work while ScalarE is still on the big e-side sin/cos
            sd, cd, d1d = trig_tiles(decproj_ps, DPC, "d")
            se, ce, d1e = trig_tiles(encproj_ps, T_ENC, "e")

            def advance(k, s, c, d1, fd, side):
                sk = tpool.tile([P, HC, fd], BF16, tag=f"s{side}", bufs=5, name=f"s{k}{side}")
                nc.vector.tensor_tensor(sk[:], d1[:], s[k - 1][:], MULT)
                nc.vector.tensor_tensor(sk[:], sk[:], s[k - 2][:], SUB)
                ck = tpool.tile([P, HC, fd], BF16, tag=f"c{side}", bufs=5, name=f"c{k}{side}")
                nc.vector.tensor_tensor(ck[:], d1[:], c[k - 1][:], MULT)
                nc.vector.tensor_tensor(ck[:], ck[:], c[k - 2][:], SUB)
                s[k] = sk
                c[k] = ck

            # ---------------- harmonic loop ----------------
            n_mm = 0
            for k in range(1, K_HARM + 1):
                if k >= 3:
                    advance(k, se, ce, d1e, T_ENC, "e")
                    advance(k, sd, cd, d1d, DPC, "d")
                # fold V*c_k into the d-side on ScalarE (idle engine):
                # per-chunk per-partition scale via activation Copy
                fs = tpool.tile([P, HC, DPC], BF16, tag="fs", bufs=4, name=f"fs{k}")
                fc = tpool.tile([P, HC, DPC], BF16, tag="fc", bufs=4, name=f"fc{k}")
                for c in range(HC):
                    nc.scalar.activation(
                        fs[:, c, :], sd[k][:, c, :], COPY, scale=Vc_sb[:, c, k - 1 : k]
                    )
                    nc.scalar.activation(
                        fc[:, c, :], cd[k][:, c, :], COPY, scale=Vc_sb[:, c, k - 1 : k]
                    )
                last = k == K_HARM
                for F, G in ((fs, ce[k]), (fc, se[k])):
                    for c in range(HC):
                        for half in range(2):
                            sl = slice(half * 512, (half + 1) * 512)
                            nc.tensor.matmul(
                                score_ps[:, sl],
                                F[:, c, :],
                                G[:, c, sl],
                                start=(n_mm == 0),
                                stop=(last and G is se[k] and c == HC - 1),
                            )
                        n_mm += 2
                # drop references no longer needed by the recurrence
                if k >= 3:
                    del se[k - 2], ce[k - 2], sd[k - 2], cd[k - 2]

            # ---------------- softmax (no max subtraction) ----------------
            expw = epool.tile([P, T_ENC], F32, tag="expw")
            sumexp = epool.tile([P, 1], F32, tag="sumexp")
            nc.scalar.activation(expw[:], score_ps[:], EXP, accum_out=sumexp[:])
            rec = epool.tile([P, 1], F32, tag="rec")
            nc.vector.reciprocal(rec[:], sumexp[:])
            wnorm = epool.tile([P, T_ENC], F32, tag="wnorm")
            nc.vector.tensor_scalar_mul(wnorm[:], expw[:], rec[:])
            nc.sync.dma_start(out=w_out_d[:], in_=wnorm[:])

            # ---------------- context = w @ enc (bf16) ----------------
            wT_sb = epool.tile([P, 8, DPC], BF16, tag="wT_sb")
            for t in range(8):
                wT_ps = paux.tile([P, P], F32, tag="aux", bufs=2, name=f"wT{t}")
                nc.tensor.transpose(
                    wT_ps[:], wnorm[:, t * P : (t + 1) * P], ident_sb[:]
                )
                nc.vector.tensor_copy(wT_sb[:, t, :], wT_ps[:])
            ctx_ps = paux.tile([P, H], F32, tag="aux", bufs=2, name="ctx_ps")
            for t in range(8):
                nc.tensor.matmul(
                    ctx_ps[:],
                    wT_sb[:, t, :],
                    enc_nat_sb[:, t, :],
                    start=(t == 0),
                    stop=(t == 7),
                )
            ctx_sb = epool.tile([P, H], F32, tag="ctx_sb")
            nc.vector.tensor_copy(ctx_sb[:], ctx_ps[:])
            nc.sync.dma_start(out=ctx_out_d[:], in_=ctx_sb[:])

    nc.compile()
    return nc


def make_in_maps(encoder_outputs, decoder_outputs, W_a, U_a, V_a):
    bf = ml_dtypes.bfloat16
    enc = np.ascontiguousarray(np.asarray(encoder_outputs, dtype=np.float32))
    dec = np.ascontiguousarray(np.asarray(decoder_outputs, dtype=np.float32))
    W = np.asarray(W_a, dtype=np.float32)
    U = np.asarray(U_a, dtype=np.float32)
    V = np.asarray(V_a, dtype=np.float32).reshape(H)

    amax = float(np.abs(dec @ U).max())
    bmax = float(np.abs(enc @ W).max())
    u0 = (amax + bmax) * 1.001
    # period: fit quality wants ~1.35*u0; Sin range safety (theta + pi/2
    # <= pi even for the cos bias) wants >= 2.2*max-side
    up = max(1.35 * u0, 2.2 * max(amax, bmax))
    omegas, coefs, fit_err = fit_sine(u0, up)

    enc_nat_all = np.ascontiguousarray(
        enc.reshape(B, 8, P, H).transpose(0, 2, 1, 3)
    ).astype(bf)
    encT_all = np.ascontiguousarray(
        enc.transpose(0, 2, 1).reshape(B, HC, P, T_ENC).transpose(0, 2, 1, 3)
    ).astype(bf)
    decT_full = dec.transpose(0, 2, 1).reshape(B, HC, P, T_DEC)
    Wr = np.ascontiguousarray(W.reshape(HC, P, HC, P).transpose(1, 0, 2, 3)).astype(bf)
    Ur = np.ascontiguousarray(U.reshape(HC, P, HC, P).transpose(1, 0, 2, 3)).astype(bf)

    Vc = np.zeros((P, HC, K_HARM), dtype=np.float32)
    for c in range(HC):
        Vc[:, c, :] = V[c * P : (c + 1) * P, None] * coefs[None, :]
    ident = np.eye(P, dtype=np.float32)

    in_maps = []
    for core in range(N_CORES):
        b, half = core // 2, core % 2
        dlo = half * DPC
        decT_core = np.ascontiguousarray(
            decT_full[b][:, :, dlo : dlo + DPC].transpose(1, 0, 2)
        ).astype(bf)
        in_maps.append(
            {
                "enc_nat": enc_nat_all[b],
                "encT": encT_all[b],
                "decT": decT_core,
                "W": Wr,
                "U": Ur,
                "Vc": Vc,
                "ident": ident,
            }
        )
    return in_maps, float(omegas[0])


def kernel(encoder_outputs, decoder_outputs, W_a, U_a, V_a):
    from concourse.bass_utils import run_bass_kernel_spmd

    in_maps, omega1 = make_in_maps(encoder_outputs, decoder_outputs, W_a, U_a, V_a)
    nc = build_graph(omega1)
    res = run_bass_kernel_spmd(nc, in_maps, core_ids=list(range(N_CORES)))

    ctx = np.zeros((B, T_DEC, H), dtype=np.float32)
    w = np.zeros((B, T_DEC, T_ENC), dtype=np.float32)
    for core in range(N_CORES):
        b, half = core // 2, core % 2
        dlo = half * DPC
        out = res.results[core]
        ctx[b, dlo : dlo + DPC] = out["ctx_out"]
        w[b, dlo : dlo + DPC] = out["w_out"]
    return ctx, w
